# revision 5
# baseline (speedup 1.0000x reference)
"""Trainium2 Bass kernel for ImportanceWeightedMoE (dense all-expert MLP).

Strategy (expert parallel, 8 cores / 8 experts):
  - Host: router (cosine-sim logits + gumbel softmax), importance softmax,
    aux losses. All O(B*T*E + E*Dh) -- negligible vs the 2.2e11 FLOP GEMMs.
  - The per-expert feature importance is folded into W1's first Dh columns,
    so every core consumes the same activation matrix
    xT = concat([h, code_emb], -1).T  [1536, 4096].
  - Core e runs a 2-GEMM MLP for expert e, feature-major end to end:
      mid[2048, 4096] = gelu(W1e'.T-contract(xT)) + b1   (PSUM fp32 accum)
      out[128, 4096]  = (W2e.T-contract(mid) + b2) * ew[:, e]
    float32r matmuls (full PE rate at N=512, near-fp32 precision).
  - Host gathers the 8 [128, 4096] slices into [B, T, 1024].
"""

import os
import numpy as np

B, T, Dh, Dc, E = 16, 256, 1024, 512, 8
S = 128           # 1024 // E
N = B * T         # 4096 tokens
K1 = Dh + Dc      # 1536
M1 = 2 * Dh       # 2048
KO1 = K1 // 128   # 12 k-tiles for GEMM1
MT1 = M1 // 128   # 16 m-tiles for GEMM1 == k-tiles for GEMM2
NBLK = 512
NB = N // NBLK    # 8 token blocks

TAU = 0.1
SCALE = 0.125
IMP_REG = 0.01

_CACHE = {}


def _build_program():
    """Build the per-core Bass/Tile program (same program on all 8 cores)."""
    import concourse.bass as bass
    import concourse.mybir as mybir
    import concourse.bacc as bacc
    import concourse.tile as tile

    f32 = mybir.dt.float32
    f32r = mybir.dt.float32r
    AF = mybir.ActivationFunctionType

    nc = bacc.Bacc(None, name="moe_expert_mlp")

    xt_d = nc.dram_tensor("xt", [128, KO1, N], f32r, kind="ExternalInput")
    w1_d = nc.dram_tensor("w1", [128, KO1, M1], f32r, kind="ExternalInput")
    w2_d = nc.dram_tensor("w2", [128, MT1, S], f32r, kind="ExternalInput")
    b1_d = nc.dram_tensor("b1", [128, MT1], f32, kind="ExternalInput")
    b2_d = nc.dram_tensor("b2", [128, 1], f32, kind="ExternalInput")
    wg_d = nc.dram_tensor("wg", [N], f32, kind="ExternalInput")
    out_d = nc.dram_tensor("out", [128, N], f32, kind="ExternalOutput")

    with tile.TileContext(nc) as tc:
        with (
            tc.tile_pool(name="weights", bufs=1) as wpool,
            tc.tile_pool(name="xin", bufs=2) as xpool,
            tc.tile_pool(name="mid", bufs=1) as mpool,
            tc.tile_pool(name="outp", bufs=2) as opool,
            tc.tile_pool(name="scales", bufs=2) as spool,
            tc.tile_pool(name="ps1", bufs=2, space="PSUM") as ps1,
            tc.tile_pool(name="ps2", bufs=2, space="PSUM") as ps2,
        ):
            # Resident weights. W1 split per k-tile so the first GEMM1
            # matmuls can start before the whole 12.6 MB has landed.
            w1_sb = []
            for k in range(KO1):
                t = wpool.tile([128, M1], f32r, tag=f"w1_{k}")
                nc.sync.dma_start(t[:], w1_d[:, k, :])
                w1_sb.append(t)
            w2_sb = wpool.tile([128, MT1, S], f32r, tag="w2")
            nc.sync.dma_start(w2_sb[:], w2_d[:])
            b1_sb = wpool.tile([128, MT1], f32, tag="b1")
            nc.sync.dma_start(b1_sb[:], b1_d[:])
            b2_sb = wpool.tile([128, 1], f32, tag="b2")
            nc.sync.dma_start(b2_sb[:], b2_d[:])

            for n in range(NB):
                nsl = slice(n * NBLK, (n + 1) * NBLK)
                xt_sb = xpool.tile([128, KO1, NBLK], f32r, tag="xt")
                nc.sync.dma_start(xt_sb[:], xt_d[:, :, nsl])
                wg_sb = spool.tile([128, NBLK], f32, tag="wg")
                nc.sync.dma_start(
                    wg_sb[:], wg_d[None, nsl].to_broadcast((128, NBLK))
                )

                mid_sb = mpool.tile([128, MT1, NBLK], f32r, tag="mid")
                for m in range(MT1):
                    pt = ps1.tile([128, NBLK], f32, tag="ps1")
                    for k in range(KO1):
                        nc.tensor.matmul(
                            pt[:],
                            w1_sb[k][:, m * 128 : (m + 1) * 128],
                            xt_sb[:, k, :],
                            start=(k == 0),
                            stop=(k == KO1 - 1),
                        )
                    # mid = gelu(psum + b1)  (exact erf gelu)
                    nc.scalar.activation(
                        mid_sb[:, m, :], pt[:], AF.Gelu,
                        bias=b1_sb[:, m : m + 1],
                    )

                pt2 = ps2.tile([128, NBLK], f32, tag="ps2")
                for m in range(MT1):
                    nc.tensor.matmul(
                        pt2[:],
                        w2_sb[:, m, :],
                        mid_sb[:, m, :],
                        start=(m == 0),
                        stop=(m == MT1 - 1),
                    )
                out_sb = opool.tile([128, NBLK], f32, tag="out")
                # out = (psum2 + b2) * expert_weight[token]
                nc.scalar.add(out_sb[:], pt2[:], b2_sb[:, 0:1])
                nc.vector.tensor_mul(out_sb[:], out_sb[:], wg_sb[:])
                nc.sync.dma_start(out_d[:, nsl], out_sb[:])

    nc.compile()
    return nc


def _host_router(code_emb, code_anchor, gumbel_u):
    """Gumbel-softmax routing weights, fp64 host math. [B, T, E]."""
    ce = code_emb.astype(np.float64)
    ca = code_anchor.astype(np.float64)
    ce_n = ce / np.maximum(np.linalg.norm(ce, axis=-1, keepdims=True), 1e-12)
    ca_n = ca / np.maximum(np.linalg.norm(ca, axis=-1, keepdims=True), 1e-12)
    logits = np.einsum("btd,ed->bte", ce_n, ca_n) * SCALE
    u = gumbel_u.astype(np.float64)
    g = -np.log(-np.log(u + 1e-10) + 1e-10)
    z = (logits + g) / TAU
    z -= z.max(-1, keepdims=True)
    ez = np.exp(z)
    return ez / ez.sum(-1, keepdims=True)


def _host_importance(feature_importance, importance_temperature):
    temp = np.clip(importance_temperature.astype(np.float64), 0.1, 5.0)
    z = feature_importance.astype(np.float64) / temp
    z = z - z.max(1, keepdims=True)
    ez = np.exp(z)
    return ez / ez.sum(1, keepdims=True)  # [E, Dh]


def _host_aux_loss(expert_weights, importance):
    ew = expert_weights  # [B, T, E] fp64
    expert_counts = ew.sum(0)  # [T, E]
    expert_load = expert_counts / (expert_counts.sum() + 1e-8)
    entropy = -(expert_load * np.log(expert_load + 1e-8)).sum()
    routing_loss = 0.5 * (expert_counts.std(ddof=1) + entropy)
    imp_ent = -(importance * np.log(importance + 1e-8)).sum(-1)  # [E]
    return np.float32(routing_loss - IMP_REG * imp_ent.mean())


def kernel(h, code_emb, code_anchor, feature_importance,
           importance_temperature, W1, b1, W2, b2, gumbel_u):
    from concourse.bass_utils import run_bass_kernel_spmd

    h = np.asarray(h, np.float32)
    code_emb = np.asarray(code_emb, np.float32)

    ew = _host_router(np.asarray(code_emb), np.asarray(code_anchor),
                      np.asarray(gumbel_u))           # [B, T, E] fp64
    imp = _host_importance(np.asarray(feature_importance),
                           np.asarray(importance_temperature))  # [E, Dh] fp64
    aux_loss = _host_aux_loss(ew, imp)
    ew32 = ew.astype(np.float32)
    imp32 = imp.astype(np.float32)

    # Shared activations: xT tiled [128, KO1, N]
    x = np.concatenate(
        [h.reshape(N, Dh), code_emb.reshape(N, Dc)], axis=1
    )  # [N, K1]
    xt3 = np.ascontiguousarray(
        x.T.reshape(KO1, 128, N).transpose(1, 0, 2)
    )  # [128, KO1, N]

    W1 = np.asarray(W1, np.float32)
    W2 = np.asarray(W2, np.float32)
    b1 = np.asarray(b1, np.float32)
    b2 = np.asarray(b2, np.float32)

    in_maps = []
    for e in range(E):
        scale = np.concatenate([imp32[e], np.ones(Dc, np.float32)])
        w1f = W1[e] * scale[None, :]                  # [M1, K1]
        w1_3 = np.ascontiguousarray(
            w1f.T.reshape(KO1, 128, M1).transpose(1, 0, 2)
        )                                             # [128, KO1, M1]
        w2_3 = np.ascontiguousarray(
            W2[e].T.reshape(MT1, 128, S).transpose(1, 0, 2)
        )                                             # [128, MT1, S]
        in_maps.append({
            "xt": xt3,
            "w1": w1_3,
            "w2": w2_3,
            "b1": np.ascontiguousarray(b1[e].reshape(MT1, 128).T),
            "b2": np.ascontiguousarray(b2[e][:, None]),
            "wg": np.ascontiguousarray(ew32[:, :, e].reshape(N)),
        })

    if "nc" not in _CACHE:
        _CACHE["nc"] = _build_program()
    nc = _CACHE["nc"]

    res = run_bass_kernel_spmd(
        nc, in_maps, core_ids=list(range(E)), trace=False,
    )
    _CACHE["last_result"] = res
    _CACHE["last_in_maps"] = in_maps

    full = np.empty((B, T, E * S), np.float32)
    for e in range(E):
        full[:, :, e * S : (e + 1) * S] = (
            res.results[e]["out"].T.reshape(B, T, S)
        )
    return full, aux_loss
